# revision 52
# baseline (speedup 1.0000x reference)
"""Trainium2 Bass kernel for BinarizeConv2d block:
   y = round(2*clip(BN(conv3x3(x, sign(w))), -1, 1))/2

Data-parallel sharding: each of 8 cores convolves 2 images over ALL 32
output channels.  BN needs full-batch stats, and NRT collectives cannot sit
inside a For_i hardware loop in this environment (verified: one CC per NEFF
execution works, CC inside For_i desyncs the mesh), so the kernel runs as
TWO loop-timeable dispatches:
  A ("stats"): conv -> raw y (fp16, partition-layout verbatim) to HBM +
     per-core BN partials [32,2]; the host adds the 8 cores' partials and
     derives scale/bias (tiny exact math, not the model itself).
  B ("final"): read raw y back, ACT affine (v = 2*bn + 2) + one DVE
     clamp-to-int8 pass (int8 store rounds RNE, matching jnp.round), one
     3-dim DMA per chunk into a d-major output layout; the host transposes
     to NCHW and maps {0..4} -> {-1,-.5,0,.5,1}.

Conv lowering (the perf core): with 32 output channels per core we use
M=128 PE columns = 32co x 4 row-phases (d).  rhs partitions hold K=96 =
32ci x 3 column-shifted copies of x (kw baked into partition groups), and
6 matmuls with row-offset rhs (a = -1..4) accumulate a [128, 224] psum
tile covering FOUR output rows: column (32d+co) gets W[co,ci,a+1-d,kw]
(zero when a+1-d is outside 0..2).  6 matmuls x 224 cycles per 4 rows vs
the old channel-sharded scheme's ~63 tiny matmuls per 2 rows: ~8x fewer
PE cycles, ~24x fewer instructions (the old kernel was issue-bound at
~47ns/matmul).  Measured floors: dispatch A is bound by the x load (3
shifted copies land on 96 partitions at ~2.1GB/s/partition -> ~95us);
dispatch B by the DVE clamp stream (~28us) plus pipeline fills.
"""
import sys
sys.path.insert(0, "/opt/trn_rl_repo")
import numpy as np
import ml_dtypes
import concourse.bass as bass
import concourse.bacc as bacc
import concourse.tile as tile
from concourse import mybir
from concourse.bass_utils import run_bass_kernel_spmd

F32 = mybir.dt.float32
F16 = mybir.dt.float16
BF16 = mybir.dt.bfloat16
I8 = mybir.dt.int8

N_CORES = 8
IMG_PC = 2        # images per core
C = 32
H = W = 224
WP = 226          # padded width (1 left + 1 right)
HP = 226          # padded height
NSLAB = 4         # row-slabs per image (56 rows each)
SR = 56
SLAB_ROWS = 58    # input rows per slab (56 + 2 halo)
SLAB_ELEMS = SLAB_ROWS * WP  # 13108
GP_SLAB = 14      # 4-row groups per slab
N_GROUPS = IMG_PC * NSLAB * GP_SLAB  # 112
MAGIC = 12582912.0  # 1.5 * 2**23 fp32 round-to-nearest-even trick
EPS = 1e-5
NTOT = float(16 * H * W)  # batch elements per channel (global)
HWs = H * W

_cache = {}
_last_ab = None     # filled by kernel(); test.py reuses them for timing B
_last_yr = None


def _build_nc(mode="final", loop_n=1, skip=(), stagger=False):
    assert mode in ("stats", "final")
    nc = bacc.Bacc("TRN2", target_bir_lowering=False, debug=False,
                   num_devices=N_CORES)
    if mode == "stats":
        xs_ext = nc.declare_dram_parameter("xs", [IMG_PC, 3, C, HP, WP], F16,
                                           isOutput=False)
        sw_ext = nc.declare_dram_parameter("sw", [96, 6, 128], F16,
                                           isOutput=False)
        sel1_ext = nc.declare_dram_parameter("sel1", [128, C], F32,
                                             isOutput=False)
        st_ext = nc.declare_dram_parameter("st", [C, 2], F32, isOutput=True)
        yr_ext = nc.declare_dram_parameter("yr", [8, 128, 14, 224], F16,
                                           isOutput=True)
        ab_ext = y_ext = None
    else:
        yr_ext = nc.declare_dram_parameter("yr", [8, 128, 14, 224], F16,
                                           isOutput=False)
        ab_ext = nc.declare_dram_parameter("ab", [128, 2], F32, isOutput=False)
        # [d, co, img, 56-row-group, w]: partition p=32d+co gets the uniform
        # stride 2*56*224, so a whole epilogue chunk writes in ONE 3-dim DMA.
        # The host reassembles NCHW (rows = 4g+d) with a cheap transpose.
        y_ext = nc.declare_dram_parameter("y", [4, C, IMG_PC, 56, W], I8,
                                          isOutput=True)
        xs_ext = sw_ext = sel1_ext = st_ext = None

    with tile.TileContext(nc) as tc:
        with (
            tc.tile_pool(name="big", bufs=1) as big,
            tc.tile_pool(name="small", bufs=1) as small,
            tc.tile_pool(name="ph2", bufs=2) as ph2,
            tc.tile_pool(name="psum", bufs=1, space="PSUM") as psum,
        ):
            # y eighth-buffers: 14 groups each (quarter image) so the
            # epilogue / writeback can stream at eighth granularity.
            yq = [big.tile([128, 14, 224], F16, name=f"yq{i}")
                  for i in range(8)]
            psum_t = psum.tile([128, 8, 512], F32)
            if mode == "stats":
                xb = [big.tile([96, SLAB_ELEMS], F16, name=f"xb{i}")
                      for i in range(3)]
                s_sb = small.tile([96, 6, 128], F16)
                stats_buf = small.tile([128, 56, 6], F32)
                sel1_sb = small.tile([128, C], F32)
                st_sb = small.tile([C, 2], F32)
                msq_scr = small.tile([128, 112], F32)
                red = small.tile([128, 4], F32)
                stats_sq = small.tile([128, 2], F32)
                nc.sync.dma_start(out=sel1_sb[:], in_=sel1_ext.ap())
                nc.sync.dma_start(out=s_sb[:], in_=sw_ext.ap())
            else:
                ab_sb = small.tile([128, 2], F32)
                nc.sync.dma_start(out=ab_sb[:], in_=ab_ext.ap())

            env = dict(locals())
            import contextlib
            loop_cm = (tc.For_i(0, loop_n, 1, staggered_reset=stagger)
                       if loop_n > 1 else contextlib.nullcontext())
            with loop_cm:
                if mode == "stats":
                    _body_stats(nc, tc, env, skip)
                else:
                    _body_final(nc, tc, env, skip)
    nc.compile()
    return nc


def _body_stats(nc, tc, env, skip=()):
    """Dispatch A: conv -> yq quarters -> yr HBM dump + bn stats -> st."""
    xb, yq, s_sb = env["xb"], env["yq"], env["s_sb"]
    stats_buf, psum_t = env["stats_buf"], env["psum_t"]
    xs_ap = env["xs_ext"].ap()
    yr_ap = env["yr_ext"].ap()

    for slab in range(IMG_PC * NSLAB):
        img, s = divmod(slab, NSLAB)
        x_c = xb[slab % 3]
        if "xdma" not in skip:
            # Two half-loads per slab on DIFFERENT HWDGE rings (SP + ACT):
            # two in-flight DMAs raise the effective per-partition fill rate
            # (whole-slab loads measured ~15% slower; a third stream via the
            # SWDGE ring regresses -- it shares the queue with yr writes),
            # and the first groups' matmuls start sooner.  (An asymmetric
            # 46/12 split of the last slab to shorten the conv tail measured
            # ~2us slower -- unbalanced halves hurt the 2-in-flight overlap.)
            h0n = 30 * WP
            for eng, (lo, cnt) in zip(
                    (nc.sync, nc.scalar),
                    ((0, h0n), (h0n, SLAB_ELEMS - h0n))):
                src = bass.AP(
                    tensor=xs_ap.tensor,
                    offset=(xs_ap.offset + img * 3 * C * HP * WP
                            + SR * s * WP + lo),
                    ap=[[C * HP * WP, 3], [HP * WP, C], [1, cnt]])
                eng.dma_start(out=x_c[0:96, lo:lo + cnt], in_=src)
        for j in range(GP_SLAB if "mm" not in skip else 0):
            g_glob = slab * GP_SLAB + j
            bank = g_glob % 8
            hl = 4 * j
            for ai in range(6):
                a = ai - 1
                off = (hl + a + 1) * WP
                nc.tensor.matmul(
                    psum_t[0:128, bank, 0:224],
                    s_sb[0:96, ai, :],
                    x_c[0:96, off:off + 224],
                    start=(ai == 0), stop=(ai == 5))
            if g_glob % 2 == 1 and "drain" not in skip:
                # drain the (even, odd) bank pair in one ACT copy
                e, qg = divmod(g_glob - 1, 14)
                pair_src = psum_t[0:128, bank - 1:bank + 1, 0:224]
                nc.scalar.copy(yq[e][0:128, qg:qg + 2, :], pair_src)
                if "stats" not in skip:
                    pair = (g_glob - 1) // 2
                    nc.vector.bn_stats(
                        out=stats_buf[0:128, pair, :],
                        in_=yq[e][0:128, qg:qg + 2, :].rearrange(
                            "p a b -> p (a b)"))
                if qg in (10, 12) and "ydma" not in skip:
                    # stream the eighth to HBM in two stages (groups 0-11
                    # after pair 10, groups 12-13 right after the last pair)
                    # so the final write tail is short
                    g_lo, g_hi = (0, 12) if qg == 10 else (12, 14)
                    dst = bass.AP(
                        tensor=yr_ap.tensor,
                        offset=(yr_ap.offset + e * 128 * 14 * 224
                                + g_lo * 224),
                        ap=[[14 * 224, 128], [1, (g_hi - g_lo) * 224]])
                    nc.gpsimd.dma_start(
                        out=dst,
                        in_=yq[e][0:128, g_lo:g_hi, :].rearrange(
                            "p a b -> p (a b)"))

    if "stats" not in skip and "drain" not in skip:
        _stats_reduce(nc, env)


def _body_final(nc, tc, env, skip=()):
    """Dispatch B: yr HBM -> yq -> affine+round+clip -> y NCHW int8."""
    yq = env["yq"]
    yr_ap = env["yr_ext"].ap()
    for e in range(8):
        if "ydma" not in skip:
            # half-loads on two HWDGE rings: 2 in-flight DMAs sustain the
            # per-partition fill rate (same effect measured on A's x loads)
            for eng, (g_lo, g_hi) in zip((nc.sync, nc.scalar),
                                         ((0, 7), (7, 14))):
                src = bass.AP(
                    tensor=yr_ap.tensor,
                    offset=yr_ap.offset + e * 128 * 14 * 224 + g_lo * 224,
                    ap=[[14 * 224, 128], [1, (g_hi - g_lo) * 224]])
                eng.dma_start(
                    out=yq[e][0:128, g_lo:g_hi, :].rearrange(
                        "p a b -> p (a b)"), in_=src)
        if "ph2" not in skip:
            _phase2_chunk(nc, env, e, 0, 14)


def _phase2_chunk(nc, env, e, g_lo, g_hi):
    """Affine+round+clip groups [g_lo, g_hi) of eighth e and DMA out."""
    yq, ab_sb = env["yq"], env["ab_sb"]
    ph2 = env["ph2"]
    y_ap = env["y_ext"].ap()
    img, ei = divmod(e, 4)          # eighth e = image img, slab ei
    ng = g_hi - g_lo
    zin = yq[e][0:128, g_lo:g_hi, :].rearrange("p a b -> p (a b)")
    n = ng * 224
    u = ph2.tile([128, n], F32, tag="u")
    nc.scalar.activation(u[:], zin,
                         mybir.ActivationFunctionType.Identity,
                         bias=ab_sb[0:128, 1:2],
                         scale=ab_sb[0:128, 0:1])
    # v = 2*bn + 2; int8 store rounds RNE (verified on hw), so
    # int8(clip(v, 0, 4.5)) == clip(round(2*bn), -2, 2) + 2 exactly.
    o = ph2.tile([128, n], I8, tag="o")
    nc.vector.tensor_scalar(o[:], u[:], 0.0, 4.5,
                            mybir.AluOpType.max, mybir.AluOpType.min)
    # groups here are rows 4g+d, g in [14*ei + g_lo, 14*ei + g_hi), d = p//32
    dst = bass.AP(
        tensor=y_ap.tensor,
        offset=y_ap.offset + img * 56 * W + (14 * ei + g_lo) * W,
        ap=[[IMG_PC * 56 * W, 128], [W, ng], [1, W]])
    nc.gpsimd.dma_start(out=dst, in_=o[:])


def _stats_reduce(nc, env):
    """stats_buf [128,56,6] -> per-channel (sum, sumsq) [32,2] -> DRAM."""
    stats_buf, psum_t = env["stats_buf"], env["psum_t"]
    msq_scr, red, stats_sq = env["msq_scr"], env["red"], env["stats_sq"]
    sel1_sb, st_sb = env["sel1_sb"], env["st_sb"]
    st_ap = env["st_ext"].ap()

    stats_fl = stats_buf.rearrange("p s (e t) -> p (s e) t", e=2, t=3)
    means = stats_fl[:, :, 1]
    ctv = stats_fl[:, :, 2]
    nc.vector.tensor_reduce(red[:, 0:1], means, mybir.AxisListType.X,
                            mybir.AluOpType.add)
    nc.vector.tensor_tensor(msq_scr[:], means, means, mybir.AluOpType.mult)
    nc.vector.tensor_reduce(red[:, 1:2], msq_scr[:], mybir.AxisListType.X,
                            mybir.AluOpType.add)
    nc.vector.tensor_reduce(red[:, 2:3], ctv, mybir.AxisListType.X,
                            mybir.AluOpType.add)
    nc.vector.tensor_scalar_mul(stats_sq[:, 0:1], red[:, 0:1], 224.0)
    nc.vector.tensor_scalar_mul(red[:, 3:4], red[:, 1:2], 224.0)
    nc.vector.tensor_tensor(stats_sq[:, 1:2], red[:, 3:4], red[:, 2:3],
                            mybir.AluOpType.add)
    nc.tensor.matmul(psum_t[0:C, 0, 0:2], sel1_sb[:], stats_sq[:],
                     start=True, stop=True)
    nc.scalar.copy(st_sb[:], psum_t[0:C, 0, 0:2])
    nc.sync.dma_start(out=st_ap, in_=st_sb[:])


def _get_nc(**kw):
    kw.pop("collective", None)
    kw.setdefault("mode", "final")
    key = tuple(sorted((k, tuple(v) if isinstance(v, (list, tuple, set)) else v)
                       for k, v in kw.items()))
    if key not in _cache:
        _cache[key] = _build_nc(**kw)
    return _cache[key]


def _prep_x(x):
    """[16,32,224,224] f32 -> per-core [2,3,32,226,226] f16 shifted copies."""
    xq = np.asarray(x, dtype=np.float32).astype(ml_dtypes.float16
                    if hasattr(ml_dtypes, "float16") else np.float16)
    xp = np.zeros((16, C, HP, WP), dtype=xq.dtype)
    xp[:, :, 1:225, 1:225] = xq
    xs3 = np.zeros((16, 3, C, HP, WP), dtype=xq.dtype)
    xs3[:, 0] = xp
    xs3[:, 1, :, :, :WP - 1] = xp[:, :, :, 1:]
    xs3[:, 2, :, :, :WP - 2] = xp[:, :, :, 2:]
    return xs3


def _prep_w(weight):
    """OIHW weight -> lhsT stack sw[96, 6, 128] f16 (binarized)."""
    w_bin = np.where(np.asarray(weight, dtype=np.float32) >= 0, 1.0,
                     -1.0).astype(np.float32)
    sw = np.zeros((96, 6, 128), dtype=np.float32)
    for ai in range(6):
        a = ai - 1
        for d in range(4):
            kh = a + 1 - d
            if 0 <= kh <= 2:
                for kw in range(3):
                    # lhsT[32*kw+ci, ai, 32*d+co] = w_bin[co, ci, kh, kw]
                    sw[32 * kw:32 * kw + 32, ai, 32 * d:32 * d + 32] = \
                        w_bin[:, :, kh, kw].T
    return sw.astype(ml_dtypes.float16
                     if hasattr(ml_dtypes, "float16") else np.float16)


def _sel1():
    p = np.arange(128)
    return (p[:, None] % 32 == np.arange(C)[None, :]).astype(np.float32)


def make_in_maps_A(x, weight):
    xs3 = _prep_x(x)
    sw = _prep_w(weight)
    sel1 = _sel1()
    return [{"xs": xs3[IMG_PC * c:IMG_PC * (c + 1)], "sw": sw, "sel1": sel1}
            for c in range(N_CORES)]


def make_in_maps_B(yr_list, ab):
    return [{"yr": yr_list[c], "ab": ab} for c in range(N_CORES)]


def reduce_stats_host(st_list, gamma, beta):
    """8x [32,2] partials -> ab [128,2] = (2*scale, 2*bias+2) replicated."""
    st = np.sum(np.stack([np.asarray(s, np.float64) for s in st_list]), axis=0)
    mean = st[:, 0] / NTOT
    var = st[:, 1] / NTOT - mean * mean
    rsq = 1.0 / np.sqrt(var + EPS)
    g = np.asarray(gamma, np.float64)
    b = np.asarray(beta, np.float64)
    scale = g * rsq
    bias = b - mean * scale
    ab32 = np.stack([2.0 * scale, 2.0 * bias + 2.0], axis=1).astype(np.float32)
    return np.tile(ab32, (4, 1))    # [128, 2], p = 32d + co


def kernel(x, weight, gamma, beta):
    global _last_ab, _last_yr
    nc_a = _get_nc(mode="stats")
    in_a = make_in_maps_A(x, weight)
    res_a = run_bass_kernel_spmd(nc_a, in_a, list(range(N_CORES)))
    ab = reduce_stats_host([res_a.results[c]["st"] for c in range(N_CORES)],
                           gamma, beta)
    yr_list = [np.asarray(res_a.results[c]["yr"]) for c in range(N_CORES)]
    _last_ab, _last_yr = ab, yr_list

    nc_b = _get_nc(mode="final")
    in_b = make_in_maps_B(yr_list, ab)
    res_b = run_bass_kernel_spmd(nc_b, in_b, list(range(N_CORES)))
    # y' [4(d), 32, 2, 56, 224] -> [2, 32, 224, 224] with rows 4g+d
    out = np.concatenate(
        [np.asarray(res_b.results[c]["y"]).transpose(2, 1, 3, 0, 4)
         .reshape(IMG_PC, C, H, W) for c in range(N_CORES)], axis=0)
    return (out.astype(np.float32) - 2.0) * 0.5


# revision 53
# speedup vs baseline: 1.0370x; 1.0370x over previous
"""Trainium2 Bass kernel for BinarizeConv2d block:
   y = round(2*clip(BN(conv3x3(x, sign(w))), -1, 1))/2

Data-parallel sharding: each of 8 cores convolves 2 images over ALL 32
output channels.  BN needs full-batch stats, and NRT collectives cannot sit
inside a For_i hardware loop in this environment (verified: one CC per NEFF
execution works, CC inside For_i desyncs the mesh), so the kernel runs as
TWO loop-timeable dispatches:
  A ("stats"): conv -> raw y (fp16, partition-layout verbatim) to HBM +
     per-core BN partials [32,2]; the host adds the 8 cores' partials and
     derives scale/bias (tiny exact math, not the model itself).
  B ("final"): read raw y back, ACT affine (v = 2*bn + 2) + one DVE
     clamp-to-int8 pass (int8 store rounds RNE, matching jnp.round), one
     3-dim DMA per chunk into a d-major output layout; the host transposes
     to NCHW and maps {0..4} -> {-1,-.5,0,.5,1}.

Conv lowering (the perf core): with 32 output channels per core we use
M=128 PE columns = 32co x 4 row-phases (d).  rhs partitions hold K=96 =
32ci x 3 column-shifted copies of x (kw baked into partition groups), and
6 matmuls with row-offset rhs (a = -1..4) accumulate a [128, 224] psum
tile covering FOUR output rows: column (32d+co) gets W[co,ci,a+1-d,kw]
(zero when a+1-d is outside 0..2).  6 matmuls x 224 cycles per 4 rows vs
the old channel-sharded scheme's ~63 tiny matmuls per 2 rows: ~8x fewer
PE cycles, ~24x fewer instructions (the old kernel was issue-bound at
~47ns/matmul).  Measured floors: dispatch A is bound by the x load (3
shifted copies land on 96 partitions at ~2.1GB/s/partition -> ~95us);
dispatch B by the DVE clamp stream (~28us) plus pipeline fills.
"""
import sys
sys.path.insert(0, "/opt/trn_rl_repo")
import numpy as np
import ml_dtypes
import concourse.bass as bass
import concourse.bacc as bacc
import concourse.tile as tile
from concourse import mybir
from concourse.bass_utils import run_bass_kernel_spmd

F32 = mybir.dt.float32
F16 = mybir.dt.float16
BF16 = mybir.dt.bfloat16
I8 = mybir.dt.int8

N_CORES = 8
IMG_PC = 2        # images per core
C = 32
H = W = 224
WP = 226          # padded width (1 left + 1 right)
HP = 226          # padded height
NSLAB = 4         # row-slabs per image (56 rows each)
SR = 56
SLAB_ROWS = 58    # input rows per slab (56 + 2 halo)
SLAB_ELEMS = SLAB_ROWS * WP  # 13108
GP_SLAB = 14      # 4-row groups per slab
N_GROUPS = IMG_PC * NSLAB * GP_SLAB  # 112
MAGIC = 12582912.0  # 1.5 * 2**23 fp32 round-to-nearest-even trick
EPS = 1e-5
NTOT = float(16 * H * W)  # batch elements per channel (global)
HWs = H * W

_cache = {}
_last_ab = None     # filled by kernel(); test.py reuses them for timing B
_last_yr = None


def _build_nc(mode="final", loop_n=1, skip=(), stagger=False):
    assert mode in ("stats", "final")
    nc = bacc.Bacc("TRN2", target_bir_lowering=False, debug=False,
                   num_devices=N_CORES)
    if mode == "stats":
        xs_ext = nc.declare_dram_parameter("xs", [IMG_PC, 3, C, HP, WP], F16,
                                           isOutput=False)
        sw_ext = nc.declare_dram_parameter("sw", [96, 6, 128], F16,
                                           isOutput=False)
        sel1_ext = nc.declare_dram_parameter("sel1", [128, C], F32,
                                             isOutput=False)
        st_ext = nc.declare_dram_parameter("st", [C, 2], F32, isOutput=True)
        yr_ext = nc.declare_dram_parameter("yr", [8, 128, 14, 224], F16,
                                           isOutput=True)
        ab_ext = y_ext = None
    else:
        yr_ext = nc.declare_dram_parameter("yr", [8, 128, 14, 224], F16,
                                           isOutput=False)
        ab_ext = nc.declare_dram_parameter("ab", [128, 2], F32, isOutput=False)
        # [d, co, img, 56-row-group, w]: partition p=32d+co gets the uniform
        # stride 2*56*224, so a whole epilogue chunk writes in ONE 3-dim DMA.
        # The host reassembles NCHW (rows = 4g+d) with a cheap transpose.
        y_ext = nc.declare_dram_parameter("y", [4, C, IMG_PC, 56, W], I8,
                                          isOutput=True)
        xs_ext = sw_ext = sel1_ext = st_ext = None

    with tile.TileContext(nc) as tc:
        with (
            tc.tile_pool(name="big", bufs=1) as big,
            tc.tile_pool(name="small", bufs=1) as small,
            tc.tile_pool(name="ph2", bufs=3) as ph2,
            tc.tile_pool(name="psum", bufs=1, space="PSUM") as psum,
        ):
            # y eighth-buffers: 14 groups each (quarter image) so the
            # epilogue / writeback can stream at eighth granularity.
            yq = [big.tile([128, 14, 224], F16, name=f"yq{i}")
                  for i in range(8)]
            psum_t = psum.tile([128, 8, 512], F32)
            if mode == "stats":
                xb = [big.tile([96, SLAB_ELEMS], F16, name=f"xb{i}")
                      for i in range(3)]
                s_sb = small.tile([96, 6, 128], F16)
                stats_buf = small.tile([128, 56, 6], F32)
                sel1_sb = small.tile([128, C], F32)
                st_sb = small.tile([C, 2], F32)
                msq_scr = small.tile([128, 112], F32)
                red = small.tile([128, 4], F32)
                stats_sq = small.tile([128, 2], F32)
                nc.sync.dma_start(out=sel1_sb[:], in_=sel1_ext.ap())
                nc.sync.dma_start(out=s_sb[:], in_=sw_ext.ap())
            else:
                ab_sb = small.tile([128, 2], F32)
                nc.sync.dma_start(out=ab_sb[:], in_=ab_ext.ap())

            env = dict(locals())
            import contextlib
            loop_cm = (tc.For_i(0, loop_n, 1, staggered_reset=stagger)
                       if loop_n > 1 else contextlib.nullcontext())
            with loop_cm:
                if mode == "stats":
                    _body_stats(nc, tc, env, skip)
                else:
                    _body_final(nc, tc, env, skip)
    nc.compile()
    return nc


def _body_stats(nc, tc, env, skip=()):
    """Dispatch A: conv -> yq quarters -> yr HBM dump + bn stats -> st."""
    xb, yq, s_sb = env["xb"], env["yq"], env["s_sb"]
    stats_buf, psum_t = env["stats_buf"], env["psum_t"]
    xs_ap = env["xs_ext"].ap()
    yr_ap = env["yr_ext"].ap()

    for slab in range(IMG_PC * NSLAB):
        img, s = divmod(slab, NSLAB)
        x_c = xb[slab % 3]
        if "xdma" not in skip:
            # Two half-loads per slab on DIFFERENT HWDGE rings (SP + ACT):
            # two in-flight DMAs raise the effective per-partition fill rate
            # (whole-slab loads measured ~15% slower; a third stream via the
            # SWDGE ring regresses -- it shares the queue with yr writes),
            # and the first groups' matmuls start sooner.  (An asymmetric
            # 46/12 split of the last slab to shorten the conv tail measured
            # ~2us slower -- unbalanced halves hurt the 2-in-flight overlap.)
            h0n = 30 * WP
            for eng, (lo, cnt) in zip(
                    (nc.sync, nc.scalar),
                    ((0, h0n), (h0n, SLAB_ELEMS - h0n))):
                src = bass.AP(
                    tensor=xs_ap.tensor,
                    offset=(xs_ap.offset + img * 3 * C * HP * WP
                            + SR * s * WP + lo),
                    ap=[[C * HP * WP, 3], [HP * WP, C], [1, cnt]])
                eng.dma_start(out=x_c[0:96, lo:lo + cnt], in_=src)
        for j in range(GP_SLAB if "mm" not in skip else 0):
            g_glob = slab * GP_SLAB + j
            bank = g_glob % 8
            hl = 4 * j
            for ai in range(6):
                a = ai - 1
                off = (hl + a + 1) * WP
                nc.tensor.matmul(
                    psum_t[0:128, bank, 0:224],
                    s_sb[0:96, ai, :],
                    x_c[0:96, off:off + 224],
                    start=(ai == 0), stop=(ai == 5))
            if g_glob % 2 == 1 and "drain" not in skip:
                # drain the (even, odd) bank pair in one ACT copy
                e, qg = divmod(g_glob - 1, 14)
                pair_src = psum_t[0:128, bank - 1:bank + 1, 0:224]
                nc.scalar.copy(yq[e][0:128, qg:qg + 2, :], pair_src)
                if "stats" not in skip:
                    pair = (g_glob - 1) // 2
                    nc.vector.bn_stats(
                        out=stats_buf[0:128, pair, :],
                        in_=yq[e][0:128, qg:qg + 2, :].rearrange(
                            "p a b -> p (a b)"))
                if qg in (10, 12) and "ydma" not in skip:
                    # stream the eighth to HBM in two stages (groups 0-11
                    # after pair 10, groups 12-13 right after the last pair)
                    # so the final write tail is short
                    g_lo, g_hi = (0, 12) if qg == 10 else (12, 14)
                    dst = bass.AP(
                        tensor=yr_ap.tensor,
                        offset=(yr_ap.offset + e * 128 * 14 * 224
                                + g_lo * 224),
                        ap=[[14 * 224, 128], [1, (g_hi - g_lo) * 224]])
                    nc.gpsimd.dma_start(
                        out=dst,
                        in_=yq[e][0:128, g_lo:g_hi, :].rearrange(
                            "p a b -> p (a b)"))

    if "stats" not in skip and "drain" not in skip:
        _stats_reduce(nc, env)


def _body_final(nc, tc, env, skip=()):
    """Dispatch B: yr HBM -> yq -> affine+round+clip -> y NCHW int8."""
    yq = env["yq"]
    yr_ap = env["yr_ext"].ap()
    for e in range(8):
        if "ydma" not in skip:
            # half-loads on two HWDGE rings: 2 in-flight DMAs sustain the
            # per-partition fill rate (same effect measured on A's x loads)
            for eng, (g_lo, g_hi) in zip((nc.sync, nc.scalar),
                                         ((0, 7), (7, 14))):
                src = bass.AP(
                    tensor=yr_ap.tensor,
                    offset=yr_ap.offset + e * 128 * 14 * 224 + g_lo * 224,
                    ap=[[14 * 224, 128], [1, (g_hi - g_lo) * 224]])
                eng.dma_start(
                    out=yq[e][0:128, g_lo:g_hi, :].rearrange(
                        "p a b -> p (a b)"), in_=src)
        if "ph2" not in skip:
            _phase2_chunk(nc, env, e, 0, 14)


def _phase2_chunk(nc, env, e, g_lo, g_hi):
    """Affine+round+clip groups [g_lo, g_hi) of eighth e and DMA out."""
    yq, ab_sb = env["yq"], env["ab_sb"]
    ph2 = env["ph2"]
    y_ap = env["y_ext"].ap()
    img, ei = divmod(e, 4)          # eighth e = image img, slab ei
    ng = g_hi - g_lo
    zin = yq[e][0:128, g_lo:g_hi, :].rearrange("p a b -> p (a b)")
    n = ng * 224
    u = ph2.tile([128, n], F32, tag="u")
    nc.scalar.activation(u[:], zin,
                         mybir.ActivationFunctionType.Identity,
                         bias=ab_sb[0:128, 1:2],
                         scale=ab_sb[0:128, 0:1])
    # v = 2*bn + 2; int8 store rounds RNE (verified on hw), so
    # int8(clip(v, 0, 4.5)) == clip(round(2*bn), -2, 2) + 2 exactly.
    o = ph2.tile([128, n], I8, tag="o")
    nc.vector.tensor_scalar(o[:], u[:], 0.0, 4.5,
                            mybir.AluOpType.max, mybir.AluOpType.min)
    # groups here are rows 4g+d, g in [14*ei + g_lo, 14*ei + g_hi), d = p//32
    dst = bass.AP(
        tensor=y_ap.tensor,
        offset=y_ap.offset + img * 56 * W + (14 * ei + g_lo) * W,
        ap=[[IMG_PC * 56 * W, 128], [W, ng], [1, W]])
    nc.gpsimd.dma_start(out=dst, in_=o[:])


def _stats_reduce(nc, env):
    """stats_buf [128,56,6] -> per-channel (sum, sumsq) [32,2] -> DRAM."""
    stats_buf, psum_t = env["stats_buf"], env["psum_t"]
    msq_scr, red, stats_sq = env["msq_scr"], env["red"], env["stats_sq"]
    sel1_sb, st_sb = env["sel1_sb"], env["st_sb"]
    st_ap = env["st_ext"].ap()

    stats_fl = stats_buf.rearrange("p s (e t) -> p (s e) t", e=2, t=3)
    means = stats_fl[:, :, 1]
    ctv = stats_fl[:, :, 2]
    nc.vector.tensor_reduce(red[:, 0:1], means, mybir.AxisListType.X,
                            mybir.AluOpType.add)
    nc.vector.tensor_tensor(msq_scr[:], means, means, mybir.AluOpType.mult)
    nc.vector.tensor_reduce(red[:, 1:2], msq_scr[:], mybir.AxisListType.X,
                            mybir.AluOpType.add)
    nc.vector.tensor_reduce(red[:, 2:3], ctv, mybir.AxisListType.X,
                            mybir.AluOpType.add)
    nc.vector.tensor_scalar_mul(stats_sq[:, 0:1], red[:, 0:1], 224.0)
    nc.vector.tensor_scalar_mul(red[:, 3:4], red[:, 1:2], 224.0)
    nc.vector.tensor_tensor(stats_sq[:, 1:2], red[:, 3:4], red[:, 2:3],
                            mybir.AluOpType.add)
    nc.tensor.matmul(psum_t[0:C, 0, 0:2], sel1_sb[:], stats_sq[:],
                     start=True, stop=True)
    nc.scalar.copy(st_sb[:], psum_t[0:C, 0, 0:2])
    nc.sync.dma_start(out=st_ap, in_=st_sb[:])


def _get_nc(**kw):
    kw.pop("collective", None)
    kw.setdefault("mode", "final")
    key = tuple(sorted((k, tuple(v) if isinstance(v, (list, tuple, set)) else v)
                       for k, v in kw.items()))
    if key not in _cache:
        _cache[key] = _build_nc(**kw)
    return _cache[key]


def _prep_x(x):
    """[16,32,224,224] f32 -> per-core [2,3,32,226,226] f16 shifted copies."""
    xq = np.asarray(x, dtype=np.float32).astype(ml_dtypes.float16
                    if hasattr(ml_dtypes, "float16") else np.float16)
    xp = np.zeros((16, C, HP, WP), dtype=xq.dtype)
    xp[:, :, 1:225, 1:225] = xq
    xs3 = np.zeros((16, 3, C, HP, WP), dtype=xq.dtype)
    xs3[:, 0] = xp
    xs3[:, 1, :, :, :WP - 1] = xp[:, :, :, 1:]
    xs3[:, 2, :, :, :WP - 2] = xp[:, :, :, 2:]
    return xs3


def _prep_w(weight):
    """OIHW weight -> lhsT stack sw[96, 6, 128] f16 (binarized)."""
    w_bin = np.where(np.asarray(weight, dtype=np.float32) >= 0, 1.0,
                     -1.0).astype(np.float32)
    sw = np.zeros((96, 6, 128), dtype=np.float32)
    for ai in range(6):
        a = ai - 1
        for d in range(4):
            kh = a + 1 - d
            if 0 <= kh <= 2:
                for kw in range(3):
                    # lhsT[32*kw+ci, ai, 32*d+co] = w_bin[co, ci, kh, kw]
                    sw[32 * kw:32 * kw + 32, ai, 32 * d:32 * d + 32] = \
                        w_bin[:, :, kh, kw].T
    return sw.astype(ml_dtypes.float16
                     if hasattr(ml_dtypes, "float16") else np.float16)


def _sel1():
    p = np.arange(128)
    return (p[:, None] % 32 == np.arange(C)[None, :]).astype(np.float32)


def make_in_maps_A(x, weight):
    xs3 = _prep_x(x)
    sw = _prep_w(weight)
    sel1 = _sel1()
    return [{"xs": xs3[IMG_PC * c:IMG_PC * (c + 1)], "sw": sw, "sel1": sel1}
            for c in range(N_CORES)]


def make_in_maps_B(yr_list, ab):
    return [{"yr": yr_list[c], "ab": ab} for c in range(N_CORES)]


def reduce_stats_host(st_list, gamma, beta):
    """8x [32,2] partials -> ab [128,2] = (2*scale, 2*bias+2) replicated."""
    st = np.sum(np.stack([np.asarray(s, np.float64) for s in st_list]), axis=0)
    mean = st[:, 0] / NTOT
    var = st[:, 1] / NTOT - mean * mean
    rsq = 1.0 / np.sqrt(var + EPS)
    g = np.asarray(gamma, np.float64)
    b = np.asarray(beta, np.float64)
    scale = g * rsq
    bias = b - mean * scale
    ab32 = np.stack([2.0 * scale, 2.0 * bias + 2.0], axis=1).astype(np.float32)
    return np.tile(ab32, (4, 1))    # [128, 2], p = 32d + co


def kernel(x, weight, gamma, beta):
    global _last_ab, _last_yr
    nc_a = _get_nc(mode="stats")
    in_a = make_in_maps_A(x, weight)
    res_a = run_bass_kernel_spmd(nc_a, in_a, list(range(N_CORES)))
    ab = reduce_stats_host([res_a.results[c]["st"] for c in range(N_CORES)],
                           gamma, beta)
    yr_list = [np.asarray(res_a.results[c]["yr"]) for c in range(N_CORES)]
    _last_ab, _last_yr = ab, yr_list

    nc_b = _get_nc(mode="final")
    in_b = make_in_maps_B(yr_list, ab)
    res_b = run_bass_kernel_spmd(nc_b, in_b, list(range(N_CORES)))
    # y' [4(d), 32, 2, 56, 224] -> [2, 32, 224, 224] with rows 4g+d
    out = np.concatenate(
        [np.asarray(res_b.results[c]["y"]).transpose(2, 1, 3, 0, 4)
         .reshape(IMG_PC, C, H, W) for c in range(N_CORES)], axis=0)
    return (out.astype(np.float32) - 2.0) * 0.5
